# revision 8
# baseline (speedup 1.0000x reference)
# Multi-head masked attention (V = Q source quirk; Wv unused) on 8 TRN2 NeuronCores.
#
# Sharding: 8 cores = 4 batches x 2 head-halves (tensor parallel). Core c
# handles batch b = c//2 and heads hh*8..hh*8+7 (hh = c%2), for ALL queries.
# Each core projects K^T and Q-natural (= V) for its own 8 heads only (no
# duplicated projection work across the pair, unlike a query split), derives
# Q^T from Q-natural via PE transposes, runs causal attention for its heads
# over all 2048 queries, then computes the PARTIAL output projection
# out^T[e, q] over its 512 hidden dims (+ bo/2). A pairwise ReduceScatter
# sums the two partials and scatters by query half, so core rank r ends with
# final out^T[:, r*1024:(r+1)*1024]. The program is SPMD-uniform: head
# assignment, Wo rows and bias live in the per-core input data, and the
# query-half selection happens inside the collective.
#
# Layouts (per core, bf16 matmul operands, fp32 PSUM accumulation):
#   kT  [128=d-in-pair, HP=4, S]   scores lhsT  (head even: partitions 0-63)
#   qT  [128=d-in-pair, HP, S]     scores rhs (from PE transposes of qn)
#   qn  [128=k-in-tile, S/128, 8*(D+1)]  attnV lhsT; col D of each head slot
#                                  is a ones column -> PSUM row 64 accumulates
#                                  the softmax denominator for free.
#   scores computed transposed (scoresT[k, q] = K @ Q^T); causal masking via
#   column-trimmed ranges + one triu multiply on the frontier 128-block.
#
# Schedule: qc-outer / head-pair-inner attention so each query chunk's attn
# completes early; projection groups and partial-outproj groups are doled out
# one per attention unit to keep the PE busy while the ACT engine (exp) is
# the per-unit latency bottleneck. ReduceScatter A (qc0|qc2) fires during the
# qc3 stream; only ReduceScatter B (qc1|qc3) and a 1MB copy are tail-serial.

import sys

for _p in ("/opt/trn_rl_repo",):
    if _p not in sys.path:
        sys.path.append(_p)

import numpy as np
import ml_dtypes

BF16 = ml_dtypes.bfloat16

B, S, E, H = 4, 2048, 1024, 16
D = E // H
NCORES = 8
NH = H // 2          # local heads per core
HP = NH // 2         # local head pairs

_CACHE = {}


def _build_program(S, E, H, n_cores=NCORES):
    import concourse.bass as bass
    import concourse.mybir as mybir
    import concourse.tile as tile
    from concourse import bacc
    from contextlib import ExitStack

    P = 128
    D = E // H
    NH = H // 2
    HP = NH // 2
    assert D == 64 and S % 512 == 0 and E % P == 0
    S_t = S // P          # seq tiles (16)
    E_t = E // P          # embed tiles (8)
    EH = NH * D           # own hidden dims (512)
    CH = 512              # q chunk
    spc = CH // P         # subtiles per chunk (4)
    n_ch = S // CH        # chunks (4)
    Lq = S // 2           # output rows per core
    f32 = mybir.dt.float32
    bf16 = mybir.dt.bfloat16
    Exp = mybir.ActivationFunctionType.Exp
    Ident = mybir.ActivationFunctionType.Identity
    scale = 1.0 / float(np.sqrt(E))
    groups = [[2 * i, 2 * i + 1] for i in range(n_cores // 2)]

    nc = bacc.Bacc(
        "TRN2", target_bir_lowering=False, debug=False, num_devices=n_cores
    )

    xT_d = nc.dram_tensor("xT", [E, S], bf16, kind="ExternalInput").ap()
    wqT_d = nc.dram_tensor("wqT", [E, EH], bf16, kind="ExternalInput").ap()
    wkT_d = nc.dram_tensor("wkT", [E, EH], bf16, kind="ExternalInput").ap()
    woT_d = nc.dram_tensor("woT", [EH, E], bf16, kind="ExternalInput").ap()
    bo_d = nc.dram_tensor("bo", [P, E_t], f32, kind="ExternalInput").ap()
    consts_d = nc.dram_tensor("consts", [P, 2 * P], bf16,
                              kind="ExternalInput").ap()
    out_d = nc.dram_tensor("out", [E, Lq], bf16, kind="ExternalOutput").ap()

    with tile.TileContext(nc) as tc, ExitStack() as ctx:
        main = ctx.enter_context(tc.tile_pool(name="main", bufs=1))
        expp = ctx.enter_context(tc.tile_pool(name="expp", bufs=3))
        stgp = ctx.enter_context(tc.tile_pool(name="stgp", bufs=2))
        ostp = ctx.enter_context(tc.tile_pool(name="ostp", bufs=3))
        dram = ctx.enter_context(tc.tile_pool(name="dram", bufs=1,
                                              space="DRAM"))

        xTs = [main.tile([P, S], bf16, tag=f"xT{e}", name=f"xT{e}")
               for e in range(E_t)]
        wqf = main.tile([P, E_t, EH], bf16)
        wkf = main.tile([P, E_t, EH], bf16)
        wo = main.tile([P, HP, E], bf16)
        qn = main.tile([P, S_t, NH * (D + 1)], bf16)
        kT = main.tile([P, HP, S], bf16)
        qT = main.tile([P, HP, S], bf16)
        attnT = main.tile([P, HP, S], bf16)
        consts = main.tile([P, 2 * P], bf16)
        bo_sb = main.tile([P, E_t], f32)
        ones128 = main.tile([1, P], bf16)
        wsrc = main.tile([P, 256], bf16)

        ba_in = dram.tile([2 * E, CH], bf16, tag="ba_in")
        bb_in = dram.tile([2 * E, CH], bf16, tag="bb_in")
        ba_out = dram.tile([E, CH], bf16, tag="ba_out")
        bb_out = dram.tile([E, CH], bf16, tag="bb_out")

        nc.vector.memset(ones128, 1.0)
        nc.vector.memset(wsrc, 0.0)

        ident = consts[:, 0:P]
        tri = consts[:, P:2 * P]
        qn4 = qn.rearrange("p t (h c) -> p t h c", c=D + 1)
        xT_r = xT_d.rearrange("(t p) s -> p t s", p=P)
        wq_r = wqT_d.rearrange("(t p) d -> p t d", p=P)
        wk_r = wkT_d.rearrange("(t p) d -> p t d", p=P)
        wo_r = woT_d.rearrange("(t p) e -> p t e", p=P)

        # ---- DMA issue order: first-needed first ----
        for e in range(E_t):
            nc.sync.dma_start(out=wqf[:, e, :], in_=wq_r[:, e, :])
        for e in range(E_t):
            nc.sync.dma_start(out=xTs[e][:, 0:CH], in_=xT_r[:, e, 0:CH])
        for e in range(E_t):
            nc.sync.dma_start(out=wkf[:, e, :], in_=wk_r[:, e, :])
        nc.sync.dma_start(out=consts, in_=consts_d)
        for sc in range(1, n_ch):
            for e in range(E_t):
                nc.sync.dma_start(
                    out=xTs[e][:, sc * CH:(sc + 1) * CH],
                    in_=xT_r[:, e, sc * CH:(sc + 1) * CH],
                )
        nc.sync.dma_start(out=bo_sb, in_=bo_d)
        for cp in range(HP):
            nc.sync.dma_start(out=wo[:, cp, :], in_=wo_r[:, cp, :])

        # ---- PE warmup: keep HAM at 8/8 while the initial DMAs stream ----
        with tc.tile_pool(name="wps", bufs=1, space="PSUM") as wpsp:
            wdst = wpsp.tile([P, 256], f32)
            for _ in range(120):
                nc.tensor.matmul(wdst, wsrc[:, 0:P], wsrc, start=True,
                                 stop=True)

        pproj = ctx.enter_context(
            tc.tile_pool(name="pproj", bufs=2, space="PSUM"))
        psc = ctx.enter_context(
            tc.tile_pool(name="psc", bufs=2, space="PSUM"))
        pav = ctx.enter_context(
            tc.tile_pool(name="pav", bufs=1, space="PSUM"))

        # ---- task groups ----
        def qn_group(st):
            ps = pproj.tile([P, CH], f32, tag="ps", name="ps")
            for e in range(E_t):
                nc.tensor.matmul(
                    ps,
                    xTs[e][:, st * P:(st + 1) * P],
                    wqf[:, e, :],
                    start=(e == 0),
                    stop=(e == E_t - 1),
                )
            nc.vector.tensor_copy(
                out=qn4[:, st, :, 0:D],
                in_=ps.rearrange("p (h c) -> p h c", c=D),
            )
            nc.gpsimd.memset(qn4[:, st, :, D:D + 1], 1.0)

        def k_group(hp, sc):
            ps = pproj.tile([P, CH], f32, tag="ps", name="ps")
            for e in range(E_t):
                nc.tensor.matmul(
                    ps,
                    wkf[:, e, hp * P:(hp + 1) * P],
                    xTs[e][:, sc * CH:(sc + 1) * CH],
                    start=(e == 0),
                    stop=(e == E_t - 1),
                )
            nc.vector.tensor_copy(out=kT[:, hp, sc * CH:(sc + 1) * CH],
                                  in_=ps)

        def t_group(hp, qc):
            # transpose qn d-blocks of 4 seq tiles into qT for one head pair
            for st in range(4 * qc, 4 * qc + 4):
                ps = pproj.tile([P, CH], f32, tag="ps", name="ps")
                pt = ps.bitcast(bf16)
                nc.tensor.transpose(
                    pt[0:D, 0:P], qn4[:, st, 2 * hp, 0:D], ident)
                nc.tensor.transpose(
                    pt[D:P, 0:P], qn4[:, st, 2 * hp + 1, 0:D], ident)
                nc.vector.tensor_copy(
                    out=qT[:, hp, st * P:(st + 1) * P], in_=pt[:, 0:P])

        def o_group(qc, et):
            # partial outproj for q-chunk qc, e-tile et, over own 4 head pairs
            ps = pproj.tile([P, CH], f32, tag="ps", name="ps")
            for cp in range(HP):
                nc.tensor.matmul(
                    ps,
                    wo[:, cp, et * P:(et + 1) * P],
                    attnT[:, cp, qc * CH:(qc + 1) * CH],
                    start=(cp == 0),
                    stop=(cp == HP - 1),
                )
            ot = ostp.tile([P, CH], bf16, tag="ot")
            nc.scalar.activation(out=ot, in_=ps, func=Ident,
                                 bias=bo_sb[:, et:et + 1])
            bnc = ba_in if qc % 2 == 0 else bb_in
            blk = qc // 2
            nc.sync.dma_start(
                out=bnc[blk * E + et * P:blk * E + (et + 1) * P, :], in_=ot)

        # ---- prefix: projections needed by the qc0 attention units ----
        for st in range(4):
            qn_group(st)
        for hp in range(HP):
            k_group(hp, 0)
        for hp in range(HP):
            t_group(hp, 0)

        # ---- attention, qc-outer ----
        pending = []

        def flush_pending(use_pav=False):
            for (php, pqc, stgs) in pending:
                for half in range(2):
                    if use_pav:
                        rb = pav.tile([P, CH], f32,
                                      tag=("pvA" if half == 0 else "pvB"),
                                      name="rb")
                    else:
                        rb = pproj.tile([P, CH], f32, tag="ps", name="rb")
                    nc.tensor.matmul(rb, ones128[0:1, :], stgs[half],
                                     start=True, stop=True)
                    rcp = stgp.tile([P, CH], f32, tag="rbs", bufs=1)
                    nc.vector.reciprocal_approx_fast(out=rcp, in_=rb)
                    dst = attnT[half * D:(half + 1) * D, php,
                                pqc * CH:(pqc + 1) * CH]
                    nc.vector.tensor_tensor(out=dst, in0=dst,
                                            in1=rcp[half * D:(half + 1) * D,
                                                    :],
                                            op=mybir.AluOpType.mult)
            pending.clear()

        rs_a_state = [False]

        def emit_rs_a():
            if rs_a_state[0]:
                return
            rs_a_state[0] = True
            # qc0's and qc2's partials are both in ba_in by now; RS_A and the
            # result copy overlap the rest of the qc3 attention stream.
            nc.gpsimd.collective_compute(
                "ReduceScatter", mybir.AluOpType.add,
                replica_groups=groups,
                ins=[ba_in.opt()], outs=[ba_out.opt()],
            )
            ocp = ostp.tile([P, E_t, CH], bf16, tag="ocp", bufs=1)
            nc.sync.dma_start(
                out=ocp, in_=ba_out.rearrange("(et p) q -> p et q", p=P))
            nc.sync.dma_start(
                out=out_d[:, 0:CH].rearrange("(et p) q -> p et q", p=P),
                in_=ocp)

        for qc in range(n_ch):
            tasks = []
            if qc + 1 < n_ch:
                for st in range(4 * qc + 4, 4 * qc + 8):
                    tasks.append(lambda st=st: qn_group(st))
                for hp in range(HP):
                    tasks.append(lambda hp=hp, q=qc + 1: k_group(hp, q))
                for hp in range(HP):
                    tasks.append(lambda hp=hp, q=qc + 1: t_group(hp, q))
            if qc >= 1:
                for et in range(E_t):
                    tasks.append(lambda et=et, q=qc - 1: o_group(q, et))
            ti = 0
            t_max = 4 * qc + 3
            tstart = 5 if qc == n_ch - 1 else 3
            slots = HP * (t_max + 1 - tstart)
            tspace = max(1, slots // max(1, len(tasks)))
            for hp in range(HP):
                hA, hB = 2 * hp, 2 * hp + 1
                pvA = pav.tile([P, CH], f32, tag="pvA")
                pvB = pav.tile([P, CH], f32, tag="pvB")
                for t in range(t_max + 1):
                    if t == 2:
                        flush_pending()
                    jloc = max(0, t - 4 * qc)
                    qoff = jloc * P
                    sc_t = psc.tile([P, 2, CH], f32, tag="sc_t")
                    nc.tensor.matmul(
                        sc_t[:, 0, qoff:CH],
                        kT[0:D, hp, t * P:(t + 1) * P],
                        qT[0:D, hp, qc * CH + qoff:(qc + 1) * CH],
                        start=True, stop=True,
                    )
                    nc.tensor.matmul(
                        sc_t[:, 1, qoff:CH],
                        kT[D:P, hp, t * P:(t + 1) * P],
                        qT[D:P, hp, qc * CH + qoff:(qc + 1) * CH],
                        start=True, stop=True,
                    )
                    ex = expp.tile([P, 2, CH], bf16)
                    nc.scalar.activation(
                        out=ex[:, :, qoff:CH],
                        in_=sc_t[:, :, qoff:CH],
                        func=Exp,
                        scale=scale,
                    )
                    if t >= 4 * qc:
                        # frontier 128-block: triu multiply (vector, NOT
                        # gpsimd: the collectives block the gpsimd queue)
                        for h2 in range(2):
                            nc.vector.tensor_mul(
                                out=ex[:, h2, qoff:qoff + P],
                                in0=ex[:, h2, qoff:qoff + P],
                                in1=tri,
                            )
                    if (t >= tstart and (t - tstart) % tspace == 0
                            and ti < len(tasks)):
                        tasks[ti]()
                        ti += 1
                        if qc == n_ch - 1 and ti == len(tasks):
                            emit_rs_a()
                    nc.tensor.matmul(
                        pvA[0:D + 1, qoff:CH],
                        qn[:, t, hA * (D + 1):(hA + 1) * (D + 1)],
                        ex[:, 0, qoff:CH],
                        start=(t == 0),
                        stop=(t == t_max),
                    )
                    nc.tensor.matmul(
                        pvB[0:D + 1, qoff:CH],
                        qn[:, t, hB * (D + 1):(hB + 1) * (D + 1)],
                        ex[:, 1, qoff:CH],
                        start=(t == 0),
                        stop=(t == t_max),
                    )
                # evict unnormalized attn + rowsum row; queue normalization
                stgs = []
                for pv, half in ((pvA, 0), (pvB, 1)):
                    stg = stgp.tile([1, CH], bf16, tag="stg", bufs=2)
                    nc.vector.tensor_copy(out=stg, in_=pv[D:D + 1, :])
                    nc.vector.tensor_copy(
                        out=attnT[half * D:(half + 1) * D, hp,
                                  qc * CH:(qc + 1) * CH],
                        in_=pv[0:D, :],
                    )
                    stgs.append(stg)
                pending.append((hp, qc, stgs))
            while ti < len(tasks):
                tasks[ti]()
                ti += 1
            if qc == n_ch - 1:
                emit_rs_a()

        # keep the PE warm across the attention->outproj transition, then
        # flush the last pending normalization into spare pav banks
        wdst2 = psc.tile([P, 2, CH], f32, tag="sc_t")
        for _ in range(4):
            nc.tensor.matmul(wdst2[:, 0, 0:256], wsrc[:, 0:P], wsrc,
                             start=True, stop=True)
        flush_pending(use_pav=True)

        # tail: qc3 partial outproj, RS_B, final copy
        for et in range(E_t):
            o_group(n_ch - 1, et)
        nc.gpsimd.collective_compute(
            "ReduceScatter", mybir.AluOpType.add,
            replica_groups=groups,
            ins=[bb_in.opt()], outs=[bb_out.opt()],
        )
        ocp2 = ostp.tile([P, E_t, CH], bf16, tag="ocp", bufs=1)
        nc.sync.dma_start(
            out=ocp2, in_=bb_out.rearrange("(et p) q -> p et q", p=P))
        nc.sync.dma_start(
            out=out_d[:, CH:2 * CH].rearrange("(et p) q -> p et q", p=P),
            in_=ocp2)

    nc.finalize()
    return nc


def _prep_inputs(x, Wk, Wq, Wo, bo, n_cores=NCORES):
    """Per-core input maps: batch = c//2, head half = c%2 (all bf16)."""
    b, s, e = x.shape
    P = 128
    EH = e // 2
    wqT = np.ascontiguousarray(Wq.T).astype(BF16)
    wkT = np.ascontiguousarray(Wk.T).astype(BF16)
    woT = np.ascontiguousarray(Wo.T).astype(BF16)
    # bias halved: both pair members add bo/2 into their partials
    bo_col = np.ascontiguousarray(
        (0.5 * bo).reshape(e // P, P).T).astype(np.float32)
    consts = np.concatenate(
        [np.eye(P, dtype=np.float32),
         np.triu(np.ones((P, P), dtype=np.float32))], axis=1).astype(BF16)
    in_maps = []
    for c in range(n_cores):
        bi, hh = c // 2, c % 2
        xT = np.ascontiguousarray(x[bi].T).astype(BF16)
        in_maps.append({
            "xT": xT,
            "wqT": np.ascontiguousarray(wqT[:, hh * EH:(hh + 1) * EH]),
            "wkT": np.ascontiguousarray(wkT[:, hh * EH:(hh + 1) * EH]),
            "woT": np.ascontiguousarray(woT[hh * EH:(hh + 1) * EH, :]),
            "bo": bo_col,
            "consts": consts,
        })
    return in_maps


def kernel(x, Wk, Wq, Wv, Wo, bo):
    from concourse import bass_utils

    x = np.asarray(x, dtype=np.float32)
    Wk = np.asarray(Wk, dtype=np.float32)
    Wq = np.asarray(Wq, dtype=np.float32)
    Wo = np.asarray(Wo, dtype=np.float32)
    bo = np.asarray(bo, dtype=np.float32)
    b, s, e = x.shape
    key = (s, e, H)
    if key not in _CACHE:
        _CACHE[key] = _build_program(s, e, H)
    nc = _CACHE[key]
    in_maps = _prep_inputs(x, Wk, Wq, Wo, bo)
    res = bass_utils.run_bass_kernel_spmd(nc, in_maps, list(range(NCORES)))
    out = np.empty((b, s, e), dtype=np.float32)
    Lq = s // 2
    for c in range(NCORES):
        bi, hh = c // 2, c % 2
        oc = np.asarray(res.results[c]["out"], dtype=np.float32)  # [E, Lq]
        out[bi, hh * Lq:(hh + 1) * Lq, :] = oc.T
    return out


if __name__ == "__main__":
    nc = _build_program(S, E, H)
    print("built ok")


# revision 12
# speedup vs baseline: 1.0979x; 1.0979x over previous
# Multi-head masked attention (V = Q source quirk; Wv unused) on 8 TRN2 NeuronCores.
#
# Sharding: 8 cores = 4 batches x 2 head-halves (tensor parallel). Core c
# handles batch b = c//2 and heads hh*8..hh*8+7 (hh = c%2), for ALL queries.
# Each core projects K^T and Q-natural (= V) for its own 8 heads only (no
# duplicated projection work across the pair, unlike a query split), derives
# Q^T from Q-natural via PE transposes, runs causal attention for its heads
# over all 2048 queries, then computes the PARTIAL output projection
# out^T[e, q] over its 512 hidden dims (+ bo/2). A pairwise ReduceScatter
# sums the two partials and scatters by query half, so core rank r ends with
# final out^T[:, r*1024:(r+1)*1024]. The program is SPMD-uniform: head
# assignment, Wo rows and bias live in the per-core input data, and the
# query-half selection happens inside the collective.
#
# Layouts (per core, bf16 matmul operands, fp32 PSUM accumulation):
#   kT  [128=d-in-pair, HP=4, S]   scores lhsT  (head even: partitions 0-63)
#   qT  [128=d-in-pair, HP, S]     scores rhs (from PE transposes of qn)
#   qn  [128=k-in-tile, S/128, 8*(D+1)]  attnV lhsT; col D of each head slot
#                                  is a ones column -> PSUM row 64 accumulates
#                                  the softmax denominator for free.
#   scores computed transposed (scoresT[k, q] = K @ Q^T); causal masking via
#   column-trimmed ranges + one triu multiply on the frontier 128-block.
#
# Schedule: qc-outer / head-pair-inner attention so each query chunk's attn
# completes early; projection groups and partial-outproj groups are doled out
# one per attention unit to keep the PE busy while the ACT engine (exp) is
# the per-unit latency bottleneck. ReduceScatter A (qc0|qc2) fires during the
# qc3 stream; only ReduceScatter B (qc1|qc3) and a 1MB copy are tail-serial.

import sys

for _p in ("/opt/trn_rl_repo",):
    if _p not in sys.path:
        sys.path.append(_p)

import numpy as np
import ml_dtypes

BF16 = ml_dtypes.bfloat16

B, S, E, H = 4, 2048, 1024, 16
D = E // H
NCORES = 8
NH = H // 2          # local heads per core
HP = NH // 2         # local head pairs

_CACHE = {}


def _build_program(S, E, H, n_cores=NCORES):
    import concourse.bass as bass
    import concourse.mybir as mybir
    import concourse.tile as tile
    from concourse import bacc
    from contextlib import ExitStack

    P = 128
    D = E // H
    NH = H // 2
    HP = NH // 2
    assert D == 64 and S % 512 == 0 and E % P == 0
    S_t = S // P          # seq tiles (16)
    E_t = E // P          # embed tiles (8)
    EH = NH * D           # own hidden dims (512)
    CH = 512              # q chunk
    spc = CH // P         # subtiles per chunk (4)
    n_ch = S // CH        # chunks (4)
    Lq = S // 2           # output rows per core
    f32 = mybir.dt.float32
    bf16 = mybir.dt.bfloat16
    Exp = mybir.ActivationFunctionType.Exp
    Ident = mybir.ActivationFunctionType.Identity
    scale = 1.0 / float(np.sqrt(E))
    groups = [[2 * i, 2 * i + 1] for i in range(n_cores // 2)]

    nc = bacc.Bacc(
        "TRN2", target_bir_lowering=False, debug=False, num_devices=n_cores
    )

    xT_d = nc.dram_tensor("xT", [E, S], bf16, kind="ExternalInput").ap()
    wqT_d = nc.dram_tensor("wqT", [E, EH], bf16, kind="ExternalInput").ap()
    wkT_d = nc.dram_tensor("wkT", [E, EH], bf16, kind="ExternalInput").ap()
    woT_d = nc.dram_tensor("woT", [EH, E], bf16, kind="ExternalInput").ap()
    bo_d = nc.dram_tensor("bo", [P, E_t], f32, kind="ExternalInput").ap()
    consts_d = nc.dram_tensor("consts", [P, 2 * P], bf16,
                              kind="ExternalInput").ap()
    out_d = nc.dram_tensor("out", [E, Lq], bf16, kind="ExternalOutput").ap()

    with tile.TileContext(nc) as tc, ExitStack() as ctx:
        main = ctx.enter_context(tc.tile_pool(name="main", bufs=1))
        expp = ctx.enter_context(tc.tile_pool(name="expp", bufs=3))
        stgp = ctx.enter_context(tc.tile_pool(name="stgp", bufs=2))
        ostp = ctx.enter_context(tc.tile_pool(name="ostp", bufs=3))
        dram = ctx.enter_context(tc.tile_pool(name="dram", bufs=1,
                                              space="DRAM"))

        xTs = [main.tile([P, S], bf16, tag=f"xT{e}", name=f"xT{e}")
               for e in range(E_t)]
        wqf = main.tile([P, E_t, EH], bf16)
        wkf = main.tile([P, E_t, EH], bf16)
        wo = main.tile([P, HP, E], bf16)
        qn = main.tile([P, S_t, NH * (D + 1)], bf16)
        kT = main.tile([P, HP, S], bf16)
        qT = main.tile([P, HP, S], bf16)
        attnT = main.tile([P, HP, S], bf16)
        consts = main.tile([P, 2 * P], bf16)
        bo_sb = main.tile([P, E_t], f32)
        ones128 = main.tile([1, P], bf16)
        wsrc = main.tile([P, 256], bf16)

        ba_in = dram.tile([2 * E, CH], bf16, tag="ba_in")
        bb_in = dram.tile([2 * E, CH], bf16, tag="bb_in")
        ba_out = dram.tile([E, CH], bf16, tag="ba_out")
        bb_out = dram.tile([E, CH], bf16, tag="bb_out")

        nc.vector.memset(ones128, 1.0)
        nc.vector.memset(wsrc, 0.0)

        ident = consts[:, 0:P]
        tri = consts[:, P:2 * P]
        qn4 = qn.rearrange("p t (h c) -> p t h c", c=D + 1)
        xT_r = xT_d.rearrange("(t p) s -> p t s", p=P)
        wq_r = wqT_d.rearrange("(t p) d -> p t d", p=P)
        wk_r = wkT_d.rearrange("(t p) d -> p t d", p=P)
        wo_r = woT_d.rearrange("(t p) e -> p t e", p=P)

        # ---- DMA issue order: first-needed first ----
        for e in range(E_t):
            nc.sync.dma_start(out=wqf[:, e, :], in_=wq_r[:, e, :])
        for e in range(E_t):
            nc.sync.dma_start(out=xTs[e][:, 0:CH], in_=xT_r[:, e, 0:CH])
        for e in range(E_t):
            nc.sync.dma_start(out=wkf[:, e, :], in_=wk_r[:, e, :])
        nc.sync.dma_start(out=consts, in_=consts_d)
        for sc in range(1, n_ch):
            for e in range(E_t):
                nc.sync.dma_start(
                    out=xTs[e][:, sc * CH:(sc + 1) * CH],
                    in_=xT_r[:, e, sc * CH:(sc + 1) * CH],
                )
        nc.sync.dma_start(out=bo_sb, in_=bo_d)
        for cp in range(HP):
            nc.sync.dma_start(out=wo[:, cp, :], in_=wo_r[:, cp, :])

        # ---- PE warmup: keep HAM at 8/8 while the initial DMAs stream ----
        with tc.tile_pool(name="wps", bufs=1, space="PSUM") as wpsp:
            wdst = wpsp.tile([P, 256], f32)
            for _ in range(120):
                nc.tensor.matmul(wdst, wsrc[:, 0:P], wsrc, start=True,
                                 stop=True)

        pproj = ctx.enter_context(
            tc.tile_pool(name="pproj", bufs=2, space="PSUM"))
        psc = ctx.enter_context(
            tc.tile_pool(name="psc", bufs=2, space="PSUM"))
        pav = ctx.enter_context(
            tc.tile_pool(name="pav", bufs=1, space="PSUM"))

        # ---- task groups ----
        def qn_group(st):
            ps = pproj.tile([P, CH], f32, tag="ps", name="ps")
            for e in range(E_t):
                nc.tensor.matmul(
                    ps,
                    xTs[e][:, st * P:(st + 1) * P],
                    wqf[:, e, :],
                    start=(e == 0),
                    stop=(e == E_t - 1),
                )
            nc.vector.tensor_copy(
                out=qn4[:, st, :, 0:D],
                in_=ps.rearrange("p (h c) -> p h c", c=D),
            )
            nc.gpsimd.memset(qn4[:, st, :, D:D + 1], 1.0)

        def k_group(hp, sc):
            ps = pproj.tile([P, CH], f32, tag="ps", name="ps")
            for e in range(E_t):
                nc.tensor.matmul(
                    ps,
                    wkf[:, e, hp * P:(hp + 1) * P],
                    xTs[e][:, sc * CH:(sc + 1) * CH],
                    start=(e == 0),
                    stop=(e == E_t - 1),
                )
            nc.vector.tensor_copy(out=kT[:, hp, sc * CH:(sc + 1) * CH],
                                  in_=ps)

        def t_group(hp, qc):
            # transpose qn d-blocks of 4 seq tiles into qT for one head pair
            for st in range(4 * qc, 4 * qc + 4):
                ps = pproj.tile([P, CH], f32, tag="ps", name="ps")
                pt = ps.bitcast(bf16)
                nc.tensor.transpose(
                    pt[0:D, 0:P], qn4[:, st, 2 * hp, 0:D], ident)
                nc.tensor.transpose(
                    pt[D:P, 0:P], qn4[:, st, 2 * hp + 1, 0:D], ident)
                nc.vector.tensor_copy(
                    out=qT[:, hp, st * P:(st + 1) * P], in_=pt[:, 0:P])

        def o_group(qc, et):
            # partial outproj for q-chunk qc, e-tile et, over own 4 head pairs
            ps = pproj.tile([P, CH], f32, tag="ps", name="ps")
            for cp in range(HP):
                nc.tensor.matmul(
                    ps,
                    wo[:, cp, et * P:(et + 1) * P],
                    attnT[:, cp, qc * CH:(qc + 1) * CH],
                    start=(cp == 0),
                    stop=(cp == HP - 1),
                )
            ot = ostp.tile([P, CH], bf16, tag="ot")
            nc.scalar.activation(out=ot, in_=ps, func=Ident,
                                 bias=bo_sb[:, et:et + 1])
            bnc = ba_in if qc % 2 == 0 else bb_in
            blk = qc // 2
            nc.sync.dma_start(
                out=bnc[blk * E + et * P:blk * E + (et + 1) * P, :], in_=ot)

        # ---- prefix: projections needed by the qc0 attention units ----
        for st in range(4):
            qn_group(st)
        for hp in range(HP):
            k_group(hp, 0)
        for hp in range(HP):
            t_group(hp, 0)

        # ---- attention, qc-outer ----
        pending = []

        def flush_pending(use_pav=False):
            for (php, pqc, stgs) in pending:
                for half in range(2):
                    if use_pav:
                        rb = pav.tile([P, CH], f32,
                                      tag=("pvA" if half == 0 else "pvB"),
                                      name="rb")
                    else:
                        rb = pproj.tile([P, CH], f32, tag="ps", name="rb")
                    nc.tensor.matmul(rb, ones128[0:1, :], stgs[half],
                                     start=True, stop=True)
                    rcp = stgp.tile([P, CH], f32, tag="rbs", bufs=1)
                    nc.vector.reciprocal_approx_fast(out=rcp, in_=rb)
                    dst = attnT[half * D:(half + 1) * D, php,
                                pqc * CH:(pqc + 1) * CH]
                    nc.vector.tensor_tensor(out=dst, in0=dst,
                                            in1=rcp[half * D:(half + 1) * D,
                                                    :],
                                            op=mybir.AluOpType.mult)
            pending.clear()

        def emit_rs_a():
            # qc0's and qc2's partials are both in ba_in by now; RS_A and the
            # result copies overlap the remaining attention streams. The
            # copies ride the gpsimd queue (ordered after the CC there) so
            # the sync queue never blocks on collective completion.
            nc.gpsimd.collective_compute(
                "ReduceScatter", mybir.AluOpType.add,
                replica_groups=groups,
                ins=[ba_in.opt()], outs=[ba_out.opt()],
            )
            ocp = ostp.tile([P, E_t, CH], bf16, tag="ocp", bufs=1)
            nc.gpsimd.dma_start(
                out=ocp, in_=ba_out.rearrange("(et p) q -> p et q", p=P))
            nc.gpsimd.dma_start(
                out=out_d[:, 0:CH].rearrange("(et p) q -> p et q", p=P),
                in_=ocp)

        # attention chunk order 0,2,1,3: RS_A's inputs (qc0|qc2) complete two
        # streams early, hiding the collective's latency entirely.
        qorder = [0, 2, 1, 3]
        stream_tasks = {
            0: ([lambda st=st: qn_group(st) for st in range(4, 16)]
                + [lambda hp=hp, sc=sc: k_group(hp, sc)
                   for sc in (1, 2) for hp in range(HP)]
                + [lambda hp=hp: t_group(hp, 2) for hp in range(HP)]),
            2: ([lambda hp=hp: t_group(hp, 1) for hp in range(HP)]
                + [lambda et=et: o_group(0, et) for et in range(E_t)]),
            1: ([lambda hp=hp: t_group(hp, 3) for hp in range(HP)]
                + [lambda hp=hp: k_group(hp, 3) for hp in range(HP)]
                + [lambda et=et: o_group(2, et) for et in range(E_t)]
                + [emit_rs_a]),
            3: [lambda et=et: o_group(1, et) for et in range(E_t)],
        }

        for qc in qorder:
            tasks = stream_tasks[qc]
            ti = 0
            t_max = 4 * qc + 3
            tstart = 5 if qc == 3 else 3
            slots = HP * max(1, t_max + 1 - tstart)
            tspace = max(1, slots // max(1, len(tasks)))
            for hp in range(HP):
                hA, hB = 2 * hp, 2 * hp + 1
                pvA = pav.tile([P, CH], f32, tag="pvA")
                pvB = pav.tile([P, CH], f32, tag="pvB")
                # software pipeline: attnV for unit t is emitted during unit
                # t+1, after the next scores+exp have been issued, so the PE
                # never sits in the scores->exp->attnV dependency chain.
                pipe = None

                def attn_v(ex, t, qoff):
                    nc.tensor.matmul(
                        pvA[0:D + 1, qoff:CH],
                        qn[:, t, hA * (D + 1):(hA + 1) * (D + 1)],
                        ex[:, 0, qoff:CH],
                        start=(t == 0),
                        stop=(t == t_max),
                    )
                    nc.tensor.matmul(
                        pvB[0:D + 1, qoff:CH],
                        qn[:, t, hB * (D + 1):(hB + 1) * (D + 1)],
                        ex[:, 1, qoff:CH],
                        start=(t == 0),
                        stop=(t == t_max),
                    )

                for t in range(t_max + 1):
                    if t == 2:
                        flush_pending()
                    jloc = max(0, t - 4 * qc)
                    qoff = jloc * P
                    sc_t = psc.tile([P, 2, CH], f32, tag="sc_t")
                    nc.tensor.matmul(
                        sc_t[:, 0, qoff:CH],
                        kT[0:D, hp, t * P:(t + 1) * P],
                        qT[0:D, hp, qc * CH + qoff:(qc + 1) * CH],
                        start=True, stop=True,
                    )
                    nc.tensor.matmul(
                        sc_t[:, 1, qoff:CH],
                        kT[D:P, hp, t * P:(t + 1) * P],
                        qT[D:P, hp, qc * CH + qoff:(qc + 1) * CH],
                        start=True, stop=True,
                    )
                    ex = expp.tile([P, 2, CH], bf16)
                    nc.scalar.activation(
                        out=ex[:, :, qoff:CH],
                        in_=sc_t[:, :, qoff:CH],
                        func=Exp,
                        scale=scale,
                    )
                    if t >= 4 * qc:
                        # frontier 128-block: triu multiply (vector, NOT
                        # gpsimd: the collectives block the gpsimd queue)
                        for h2 in range(2):
                            nc.vector.tensor_mul(
                                out=ex[:, h2, qoff:qoff + P],
                                in0=ex[:, h2, qoff:qoff + P],
                                in1=tri,
                            )
                    if pipe is not None:
                        attn_v(*pipe)
                    pipe = (ex, t, qoff)
                    if (t >= tstart and (t - tstart) % tspace == 0
                            and ti < len(tasks)):
                        tasks[ti]()
                        ti += 1
                attn_v(*pipe)
                # evict unnormalized attn + rowsum row; queue normalization
                stgs = []
                for pv, half in ((pvA, 0), (pvB, 1)):
                    stg = stgp.tile([1, CH], bf16, tag="stg", bufs=2)
                    nc.vector.tensor_copy(out=stg, in_=pv[D:D + 1, :])
                    nc.vector.tensor_copy(
                        out=attnT[half * D:(half + 1) * D, hp,
                                  qc * CH:(qc + 1) * CH],
                        in_=pv[0:D, :],
                    )
                    stgs.append(stg)
                pending.append((hp, qc, stgs))
            while ti < len(tasks):
                tasks[ti]()
                ti += 1

        # keep the PE warm across the attention->outproj transition, then
        # flush the last pending normalization into spare pav banks
        wdst2 = psc.tile([P, 2, CH], f32, tag="sc_t")
        for _ in range(4):
            nc.tensor.matmul(wdst2[:, 0, 0:256], wsrc[:, 0:P], wsrc,
                             start=True, stop=True)
        flush_pending(use_pav=True)

        # tail: qc3 partial outproj, RS_B, final copy
        for et in range(E_t):
            o_group(n_ch - 1, et)
        nc.gpsimd.collective_compute(
            "ReduceScatter", mybir.AluOpType.add,
            replica_groups=groups,
            ins=[bb_in.opt()], outs=[bb_out.opt()],
        )
        ocp2 = ostp.tile([P, E_t, CH], bf16, tag="ocp", bufs=1)
        nc.sync.dma_start(
            out=ocp2, in_=bb_out.rearrange("(et p) q -> p et q", p=P))
        nc.sync.dma_start(
            out=out_d[:, CH:2 * CH].rearrange("(et p) q -> p et q", p=P),
            in_=ocp2)

    nc.finalize()
    return nc


def _prep_inputs(x, Wk, Wq, Wo, bo, n_cores=NCORES):
    """Per-core input maps: batch = c//2, head half = c%2 (all bf16)."""
    b, s, e = x.shape
    P = 128
    EH = e // 2
    wqT = np.ascontiguousarray(Wq.T).astype(BF16)
    wkT = np.ascontiguousarray(Wk.T).astype(BF16)
    woT = np.ascontiguousarray(Wo.T).astype(BF16)
    # bias halved: both pair members add bo/2 into their partials
    bo_col = np.ascontiguousarray(
        (0.5 * bo).reshape(e // P, P).T).astype(np.float32)
    consts = np.concatenate(
        [np.eye(P, dtype=np.float32),
         np.triu(np.ones((P, P), dtype=np.float32))], axis=1).astype(BF16)
    in_maps = []
    for c in range(n_cores):
        bi, hh = c // 2, c % 2
        xT = np.ascontiguousarray(x[bi].T).astype(BF16)
        in_maps.append({
            "xT": xT,
            "wqT": np.ascontiguousarray(wqT[:, hh * EH:(hh + 1) * EH]),
            "wkT": np.ascontiguousarray(wkT[:, hh * EH:(hh + 1) * EH]),
            "woT": np.ascontiguousarray(woT[hh * EH:(hh + 1) * EH, :]),
            "bo": bo_col,
            "consts": consts,
        })
    return in_maps


def kernel(x, Wk, Wq, Wv, Wo, bo):
    from concourse import bass_utils

    x = np.asarray(x, dtype=np.float32)
    Wk = np.asarray(Wk, dtype=np.float32)
    Wq = np.asarray(Wq, dtype=np.float32)
    Wo = np.asarray(Wo, dtype=np.float32)
    bo = np.asarray(bo, dtype=np.float32)
    b, s, e = x.shape
    key = (s, e, H)
    if key not in _CACHE:
        _CACHE[key] = _build_program(s, e, H)
    nc = _CACHE[key]
    in_maps = _prep_inputs(x, Wk, Wq, Wo, bo)
    res = bass_utils.run_bass_kernel_spmd(nc, in_maps, list(range(NCORES)))
    out = np.empty((b, s, e), dtype=np.float32)
    Lq = s // 2
    for c in range(NCORES):
        bi, hh = c // 2, c % 2
        oc = np.asarray(res.results[c]["out"], dtype=np.float32)  # [E, Lq]
        out[bi, hh * Lq:(hh + 1) * Lq, :] = oc.T
    return out


if __name__ == "__main__":
    nc = _build_program(S, E, H)
    print("built ok")


# revision 20
# speedup vs baseline: 1.1581x; 1.0548x over previous
# Multi-head masked attention (V = Q source quirk; Wv unused) on 8 TRN2 NeuronCores.
#
# Sharding: 8 cores = 4 batches x 2 head-halves (tensor parallel). Core c
# handles batch b = c//2 and heads hh*8..hh*8+7 (hh = c%2), for ALL queries.
# Each core projects K^T and Q-natural (= V) for its own 8 heads only (no
# duplicated projection work across the pair, unlike a query split), derives
# Q^T from Q-natural via PE transposes, runs causal attention for its heads
# over all 2048 queries, then computes the PARTIAL output projection
# out^T[e, q] over its 512 hidden dims (+ bo/2). A pairwise ReduceScatter
# sums the two partials and scatters by query half, so core rank r ends with
# final out^T[:, r*1024:(r+1)*1024]. The program is SPMD-uniform: head
# assignment, Wo rows and bias live in the per-core input data, and the
# query-half selection happens inside the collective.
#
# Layouts (per core, bf16 matmul operands, fp32 PSUM accumulation):
#   kT  [128=d-in-pair, HP=4, S]   scores lhsT  (head even: partitions 0-63)
#   qT  [128=d-in-pair, HP, S]     scores rhs (from PE transposes of qn)
#   qn  [128=k-in-tile, S/128, 8*(D+1)]  attnV lhsT; col D of each head slot
#                                  is a ones column -> PSUM row 64 accumulates
#                                  the softmax denominator for free.
#   scores computed transposed (scoresT[k, q] = K @ Q^T); causal masking via
#   column-trimmed ranges + one triu multiply on the frontier 128-block.
#
# Schedule: qc-outer / head-pair-inner attention so each query chunk's attn
# completes early; projection groups and partial-outproj groups are doled out
# one per attention unit to keep the PE busy while the ACT engine (exp) is
# the per-unit latency bottleneck. ReduceScatter A (qc0|qc2) fires during the
# qc3 stream; only ReduceScatter B (qc1|qc3) and a 1MB copy are tail-serial.

import sys

for _p in ("/opt/trn_rl_repo",):
    if _p not in sys.path:
        sys.path.append(_p)

import numpy as np
import ml_dtypes

BF16 = ml_dtypes.bfloat16

B, S, E, H = 4, 2048, 1024, 16
D = E // H
NCORES = 8
NH = H // 2          # local heads per core
HP = NH // 2         # local head pairs

_CACHE = {}


def _build_program(S, E, H, n_cores=NCORES):
    import concourse.bass as bass
    import concourse.mybir as mybir
    import concourse.tile as tile
    from concourse import bacc
    from contextlib import ExitStack

    P = 128
    D = E // H
    NH = H // 2
    HP = NH // 2
    assert D == 64 and S % 512 == 0 and E % P == 0
    S_t = S // P          # seq tiles (16)
    E_t = E // P          # embed tiles (8)
    EH = NH * D           # own hidden dims (512)
    CH = 512              # q chunk
    spc = CH // P         # subtiles per chunk (4)
    n_ch = S // CH        # chunks (4)
    Lq = S // 2           # output rows per core
    f32 = mybir.dt.float32
    bf16 = mybir.dt.bfloat16
    Exp = mybir.ActivationFunctionType.Exp
    Ident = mybir.ActivationFunctionType.Identity
    scale = 1.0 / float(np.sqrt(E))
    groups = [[2 * i, 2 * i + 1] for i in range(n_cores // 2)]

    nc = bacc.Bacc(
        "TRN2", target_bir_lowering=False, debug=False, num_devices=n_cores
    )

    xT_d = nc.dram_tensor("xT", [E, S], bf16, kind="ExternalInput").ap()
    wqT_d = nc.dram_tensor("wqT", [E, EH], bf16, kind="ExternalInput").ap()
    wkT_d = nc.dram_tensor("wkT", [E, EH], bf16, kind="ExternalInput").ap()
    # full hidden rows x my 512 output columns (e-split output projection)
    woT_d = nc.dram_tensor("woT", [E, EH], bf16, kind="ExternalInput").ap()
    bo_d = nc.dram_tensor("bo", [P, EH // P], f32, kind="ExternalInput").ap()
    consts_d = nc.dram_tensor("consts", [P, 2 * P], bf16,
                              kind="ExternalInput").ap()
    # transposed output: my 512 e-columns for ALL queries
    out_d = nc.dram_tensor("out", [EH, S], bf16, kind="ExternalOutput").ap()

    with tile.TileContext(nc) as tc, ExitStack() as ctx:
        main = ctx.enter_context(tc.tile_pool(name="main", bufs=1))
        expp = ctx.enter_context(tc.tile_pool(name="expp", bufs=3))
        stgp = ctx.enter_context(tc.tile_pool(name="stgp", bufs=2))
        ostp = ctx.enter_context(tc.tile_pool(name="ostp", bufs=3))
        dram = ctx.enter_context(tc.tile_pool(name="dram", bufs=1,
                                              space="DRAM"))

        xTs = [main.tile([P, S], bf16, tag=f"xT{e}", name=f"xT{e}")
               for e in range(E_t)]
        wqf = main.tile([P, E_t, EH], bf16)
        wkf = main.tile([P, E_t, EH], bf16)
        wo = main.tile([P, E_t, EH], bf16)
        qn = main.tile([P, S_t, NH * (D + 1)], bf16)
        kT = main.tile([P, HP, S], bf16)
        qT = main.tile([P, HP, S], bf16)
        attnT = main.tile([P, HP, S], bf16)
        # all 16 heads' attn (own + peer, head order 0..15), via AllGather
        attnF = main.tile([P, 2 * HP, S], bf16)
        consts = main.tile([P, 2 * P], bf16)
        bo_sb = main.tile([P, EH // P], f32)
        ones128 = main.tile([1, P], bf16)
        wsrc = main.tile([P, 256], bf16)

        bxi = [dram.tile([HP * P, CH], bf16, tag=f"bxi{qc}", name=f"bxi{qc}")
               for qc in range(n_ch)]
        bxo = [dram.tile([2 * HP * P, CH], bf16, tag=f"bxo{qc}",
                         name=f"bxo{qc}") for qc in range(n_ch)]

        nc.vector.memset(ones128, 1.0)
        nc.vector.memset(wsrc, 0.0)

        ident = consts[:, 0:P]
        tri = consts[:, P:2 * P]
        qn4 = qn.rearrange("p t (h c) -> p t h c", c=D + 1)
        xT_r = xT_d.rearrange("(t p) s -> p t s", p=P)
        wq_r = wqT_d.rearrange("(t p) d -> p t d", p=P)
        wk_r = wkT_d.rearrange("(t p) d -> p t d", p=P)
        wo_r = woT_d.rearrange("(t p) e -> p t e", p=P)

        # ---- DMA issue order: first-needed first ----
        for e in range(E_t):
            nc.sync.dma_start(out=wqf[:, e, :], in_=wq_r[:, e, :])
        for e in range(E_t):
            nc.sync.dma_start(out=xTs[e][:, 0:CH], in_=xT_r[:, e, 0:CH])
        for e in range(E_t):
            nc.sync.dma_start(out=wkf[:, e, :], in_=wk_r[:, e, :])
        nc.sync.dma_start(out=consts, in_=consts_d)
        for sc in range(1, n_ch):
            for e in range(E_t):
                nc.sync.dma_start(
                    out=xTs[e][:, sc * CH:(sc + 1) * CH],
                    in_=xT_r[:, e, sc * CH:(sc + 1) * CH],
                )
        nc.sync.dma_start(out=bo_sb, in_=bo_d)
        for cp in range(E_t):
            nc.sync.dma_start(out=wo[:, cp, :], in_=wo_r[:, cp, :])

        # ---- PE warmup: keep HAM at 8/8 while the initial DMAs stream ----
        with tc.tile_pool(name="wps", bufs=1, space="PSUM") as wpsp:
            wdst = wpsp.tile([P, 256], f32)
            for _ in range(80):
                nc.tensor.matmul(wdst, wsrc[:, 0:P], wsrc, start=True,
                                 stop=True)

        pproj = ctx.enter_context(
            tc.tile_pool(name="pproj", bufs=2, space="PSUM"))
        psc = ctx.enter_context(
            tc.tile_pool(name="psc", bufs=2, space="PSUM"))
        pav = ctx.enter_context(
            tc.tile_pool(name="pav", bufs=1, space="PSUM"))

        # ---- task groups ----
        def qn_group(st):
            ps = pproj.tile([P, CH], f32, tag="ps", name="ps")
            for e in range(E_t):
                nc.tensor.matmul(
                    ps,
                    xTs[e][:, st * P:(st + 1) * P],
                    wqf[:, e, :],
                    start=(e == 0),
                    stop=(e == E_t - 1),
                )
            nc.vector.tensor_copy(
                out=qn4[:, st, :, 0:D],
                in_=ps.rearrange("p (h c) -> p h c", c=D),
            )
            nc.gpsimd.memset(qn4[:, st, :, D:D + 1], 1.0)

        def k_group(hp, sc):
            ps = pproj.tile([P, CH], f32, tag="ps", name="ps")
            for e in range(E_t):
                nc.tensor.matmul(
                    ps,
                    wkf[:, e, hp * P:(hp + 1) * P],
                    xTs[e][:, sc * CH:(sc + 1) * CH],
                    start=(e == 0),
                    stop=(e == E_t - 1),
                )
            nc.vector.tensor_copy(out=kT[:, hp, sc * CH:(sc + 1) * CH],
                                  in_=ps)

        def t_group(hp, qc):
            # transpose qn d-blocks of 4 seq tiles into qT for one head pair
            for st in range(4 * qc, 4 * qc + 4):
                ps = pproj.tile([P, CH], f32, tag="ps", name="ps")
                pt = ps.bitcast(bf16)
                nc.tensor.transpose(
                    pt[0:D, 0:P], qn4[:, st, 2 * hp, 0:D], ident)
                nc.tensor.transpose(
                    pt[D:P, 0:P], qn4[:, st, 2 * hp + 1, 0:D], ident)
                nc.vector.tensor_copy(
                    out=qT[:, hp, st * P:(st + 1) * P], in_=pt[:, 0:P])

        def x_group(qc):
            # exchange q-chunk qc of attnT with the pair peer via AllGather;
            # both contributions land in attnF in fixed head order 0..15, so
            # the program stays uniform. Post-CC DMAs ride the gpsimd queue.
            nc.sync.dma_start(
                out=bxi[qc].rearrange("(hp p) q -> p hp q", p=P),
                in_=attnT[:, :, qc * CH:(qc + 1) * CH])
            nc.gpsimd.collective_compute(
                "AllGather", mybir.AluOpType.bypass,
                replica_groups=groups,
                ins=[bxi[qc].opt()], outs=[bxo[qc].opt()],
            )
            nc.gpsimd.dma_start(
                out=attnF[:, :, qc * CH:(qc + 1) * CH],
                in_=bxo[qc].rearrange("(s p) q -> p s q", p=P))

        def o_group(qc, et):
            # output projection for q-chunk qc, e-tile et of my 512 columns,
            # over all 16 heads (attnF), bias added on the ACT-engine evict
            ps = pproj.tile([P, CH], f32, tag="ps", name="ps")
            for cp in range(2 * HP):
                nc.tensor.matmul(
                    ps,
                    wo[:, cp, et * P:(et + 1) * P],
                    attnF[:, cp, qc * CH:(qc + 1) * CH],
                    start=(cp == 0),
                    stop=(cp == 2 * HP - 1),
                )
            ot = ostp.tile([P, CH], bf16, tag="ot")
            nc.scalar.activation(out=ot, in_=ps, func=Ident,
                                 bias=bo_sb[:, et:et + 1])
            nc.sync.dma_start(
                out=out_d[et * P:(et + 1) * P, qc * CH:(qc + 1) * CH],
                in_=ot)

        # ---- prefix: projections needed by the qc0 attention units ----
        for st in range(4):
            qn_group(st)
        for hp in range(HP):
            k_group(hp, 0)
        for hp in range(HP):
            t_group(hp, 0)

        # ---- attention, qc-outer ----
        pending = []

        def flush_pending(use_pav=False):
            for (php, pqc, stgs) in pending:
                for half in range(2):
                    if use_pav:
                        rb = pav.tile([P, CH], f32,
                                      tag=("pvA" if half == 0 else "pvB"),
                                      name="rb")
                    else:
                        rb = pproj.tile([P, CH], f32, tag="ps", name="rb")
                    nc.tensor.matmul(rb, ones128[0:1, :], stgs[half],
                                     start=True, stop=True)
                    rcp = stgp.tile([P, CH], f32, tag="rbs", bufs=1)
                    nc.vector.reciprocal_approx_fast(out=rcp, in_=rb)
                    dst = attnT[half * D:(half + 1) * D, php,
                                pqc * CH:(pqc + 1) * CH]
                    nc.vector.tensor_tensor(out=dst, in0=dst,
                                            in1=rcp[half * D:(half + 1) * D,
                                                    :],
                                            op=mybir.AluOpType.mult)
            pending.clear()

        # attention chunk order 0,2,1,3: each chunk's AllGather exchange is
        # emitted one stream after its normalization and consumed (by the
        # output projection) one stream later still, hiding the ~16us
        # collective latency; only qc3's exchange is tail-serial.
        qorder = [0, 2, 1, 3]
        NE = EH // P  # my output e-tiles (4)
        stream_tasks = {
            0: ([lambda st=st: qn_group(st) for st in range(4, 16)]
                + [lambda hp=hp, sc=sc: k_group(hp, sc)
                   for sc in (1, 2) for hp in range(HP)]
                + [lambda hp=hp: t_group(hp, 2) for hp in range(HP)]),
            2: ([lambda: x_group(0)]
                + [lambda hp=hp: t_group(hp, 1) for hp in range(HP)]),
            1: ([lambda: x_group(2)]
                + [lambda hp=hp: t_group(hp, 3) for hp in range(HP)]
                + [lambda hp=hp: k_group(hp, 3) for hp in range(HP)]
                + [lambda et=et: o_group(0, et) for et in range(NE)]),
            3: ([lambda: x_group(1)]
                + [lambda et=et: o_group(2, et) for et in range(NE)]
                + [lambda et=et: o_group(1, et) for et in range(NE)]),
        }

        for qc in qorder:
            tasks = stream_tasks[qc]
            ti = 0
            t_max = 4 * qc + 3
            tstart = 5 if qc == 3 else 3
            slots = HP * max(1, t_max + 1 - tstart)
            tspace = max(1, slots // max(1, len(tasks)))
            for hp in range(HP):
                hA, hB = 2 * hp, 2 * hp + 1
                pvA = pav.tile([P, CH], f32, tag="pvA")
                pvB = pav.tile([P, CH], f32, tag="pvB")
                # software pipeline: attnV for unit t is emitted during unit
                # t+1, after the next scores+exp have been issued, so the PE
                # never sits in the scores->exp->attnV dependency chain.
                pipe = None

                def attn_v(ex, t, qoff):
                    nc.tensor.matmul(
                        pvA[0:D + 1, qoff:CH],
                        qn[:, t, hA * (D + 1):(hA + 1) * (D + 1)],
                        ex[:, 0, qoff:CH],
                        start=(t == 0),
                        stop=(t == t_max),
                    )
                    nc.tensor.matmul(
                        pvB[0:D + 1, qoff:CH],
                        qn[:, t, hB * (D + 1):(hB + 1) * (D + 1)],
                        ex[:, 1, qoff:CH],
                        start=(t == 0),
                        stop=(t == t_max),
                    )

                for t in range(t_max + 1):
                    if t == 2:
                        flush_pending()
                    jloc = max(0, t - 4 * qc)
                    qoff = jloc * P
                    sc_t = psc.tile([P, 2, CH], f32, tag="sc_t")
                    nc.tensor.matmul(
                        sc_t[:, 0, qoff:CH],
                        kT[0:D, hp, t * P:(t + 1) * P],
                        qT[0:D, hp, qc * CH + qoff:(qc + 1) * CH],
                        start=True, stop=True,
                    )
                    nc.tensor.matmul(
                        sc_t[:, 1, qoff:CH],
                        kT[D:P, hp, t * P:(t + 1) * P],
                        qT[D:P, hp, qc * CH + qoff:(qc + 1) * CH],
                        start=True, stop=True,
                    )
                    ex = expp.tile([P, 2, CH], bf16)
                    nc.scalar.activation(
                        out=ex[:, :, qoff:CH],
                        in_=sc_t[:, :, qoff:CH],
                        func=Exp,
                        scale=scale,
                    )
                    if t >= 4 * qc:
                        # frontier 128-block: triu multiply (vector, NOT
                        # gpsimd: the collectives block the gpsimd queue)
                        for h2 in range(2):
                            nc.vector.tensor_mul(
                                out=ex[:, h2, qoff:qoff + P],
                                in0=ex[:, h2, qoff:qoff + P],
                                in1=tri,
                            )
                    if pipe is not None:
                        attn_v(*pipe)
                    pipe = (ex, t, qoff)
                    if (t >= tstart and (t - tstart) % tspace == 0
                            and ti < len(tasks)):
                        tasks[ti]()
                        ti += 1
                attn_v(*pipe)
                # evict unnormalized attn + rowsum row; queue normalization
                stgs = []
                for pv, half in ((pvA, 0), (pvB, 1)):
                    stg = stgp.tile([1, CH], bf16, tag="stg", bufs=2)
                    nc.vector.tensor_copy(out=stg, in_=pv[D:D + 1, :])
                    nc.vector.tensor_copy(
                        out=attnT[half * D:(half + 1) * D, hp,
                                  qc * CH:(qc + 1) * CH],
                        in_=pv[0:D, :],
                    )
                    stgs.append(stg)
                pending.append((hp, qc, stgs))
            while ti < len(tasks):
                tasks[ti]()
                ti += 1

        # flush the last pending normalization into spare pav banks, then
        # tail: qc3 exchange + its output projection
        wdst2 = psc.tile([P, 2, CH], f32, tag="sc_t")
        for _ in range(4):
            nc.tensor.matmul(wdst2[:, 0, 0:256], wsrc[:, 0:P], wsrc,
                             start=True, stop=True)
        flush_pending(use_pav=True)
        x_group(n_ch - 1)
        for et in range(EH // P):
            o_group(n_ch - 1, et)

    nc.finalize()
    return nc


def _prep_inputs(x, Wk, Wq, Wo, bo, n_cores=NCORES):
    """Per-core input maps: batch = c//2, head half = c%2 (all bf16).

    wq/wk columns select the core's 8 heads; wo columns select the core's
    512 OUTPUT dims (e-split outproj over all 16 heads via the exchange).
    """
    b, s, e = x.shape
    P = 128
    EH = e // 2
    wqT = np.ascontiguousarray(Wq.T).astype(BF16)
    wkT = np.ascontiguousarray(Wk.T).astype(BF16)
    woT = np.ascontiguousarray(Wo.T).astype(BF16)
    consts = np.concatenate(
        [np.eye(P, dtype=np.float32),
         np.triu(np.ones((P, P), dtype=np.float32))], axis=1).astype(BF16)
    in_maps = []
    for c in range(n_cores):
        bi, hh = c // 2, c % 2
        xT = np.ascontiguousarray(x[bi].T).astype(BF16)
        bo_col = np.ascontiguousarray(
            bo[hh * EH:(hh + 1) * EH].reshape(EH // P, P).T
        ).astype(np.float32)
        in_maps.append({
            "xT": xT,
            "wqT": np.ascontiguousarray(wqT[:, hh * EH:(hh + 1) * EH]),
            "wkT": np.ascontiguousarray(wkT[:, hh * EH:(hh + 1) * EH]),
            "woT": np.ascontiguousarray(woT[:, hh * EH:(hh + 1) * EH]),
            "bo": bo_col,
            "consts": consts,
        })
    return in_maps


def kernel(x, Wk, Wq, Wv, Wo, bo):
    from concourse import bass_utils

    x = np.asarray(x, dtype=np.float32)
    Wk = np.asarray(Wk, dtype=np.float32)
    Wq = np.asarray(Wq, dtype=np.float32)
    Wo = np.asarray(Wo, dtype=np.float32)
    bo = np.asarray(bo, dtype=np.float32)
    b, s, e = x.shape
    key = (s, e, H)
    if key not in _CACHE:
        _CACHE[key] = _build_program(s, e, H)
    nc = _CACHE[key]
    in_maps = _prep_inputs(x, Wk, Wq, Wo, bo)
    res = bass_utils.run_bass_kernel_spmd(nc, in_maps, list(range(NCORES)))
    out = np.empty((b, s, e), dtype=np.float32)
    EH = e // 2
    for c in range(NCORES):
        bi, hh = c // 2, c % 2
        oc = np.asarray(res.results[c]["out"], dtype=np.float32)  # [EH, S]
        out[bi, :, hh * EH:(hh + 1) * EH] = oc.T
    return out


if __name__ == "__main__":
    nc = _build_program(S, E, H)
    print("built ok")


# revision 23
# speedup vs baseline: 1.1783x; 1.0174x over previous
# Multi-head masked attention (V = Q source quirk; Wv unused) on 8 TRN2 NeuronCores.
#
# Sharding: 8 cores = 4 batches x 2 head-halves (tensor parallel). Core c
# handles batch b = c//2 and heads hh*8..hh*8+7 (hh = c%2), for ALL queries.
# Each core projects K^T and Q-natural (= V) for its own 8 heads only (no
# duplicated projection work across the pair, unlike a query split), derives
# Q^T from Q-natural via PE transposes, runs causal attention for its heads
# over all 2048 queries, then computes the PARTIAL output projection
# out^T[e, q] over its 512 hidden dims (+ bo/2). A pairwise ReduceScatter
# sums the two partials and scatters by query half, so core rank r ends with
# final out^T[:, r*1024:(r+1)*1024]. The program is SPMD-uniform: head
# assignment, Wo rows and bias live in the per-core input data, and the
# query-half selection happens inside the collective.
#
# Layouts (per core, bf16 matmul operands, fp32 PSUM accumulation):
#   kT  [128=d-in-pair, HP=4, S]   scores lhsT  (head even: partitions 0-63)
#   qT  [128=d-in-pair, HP, S]     scores rhs (from PE transposes of qn)
#   qn  [128=k-in-tile, S/128, 8*(D+1)]  attnV lhsT; col D of each head slot
#                                  is a ones column -> PSUM row 64 accumulates
#                                  the softmax denominator for free.
#   scores computed transposed (scoresT[k, q] = K @ Q^T); causal masking via
#   column-trimmed ranges + one triu multiply on the frontier 128-block.
#
# Schedule: qc-outer / head-pair-inner attention so each query chunk's attn
# completes early; projection groups and partial-outproj groups are doled out
# one per attention unit to keep the PE busy while the ACT engine (exp) is
# the per-unit latency bottleneck. ReduceScatter A (qc0|qc2) fires during the
# qc3 stream; only ReduceScatter B (qc1|qc3) and a 1MB copy are tail-serial.

import sys

for _p in ("/opt/trn_rl_repo",):
    if _p not in sys.path:
        sys.path.append(_p)

import numpy as np
import ml_dtypes

BF16 = ml_dtypes.bfloat16

B, S, E, H = 4, 2048, 1024, 16
D = E // H
NCORES = 8
NH = H // 2          # local heads per core
HP = NH // 2         # local head pairs

_CACHE = {}


def _build_program(S, E, H, n_cores=NCORES):
    import concourse.bass as bass
    import concourse.mybir as mybir
    import concourse.tile as tile
    from concourse import bacc
    from contextlib import ExitStack

    P = 128
    D = E // H
    NH = H // 2
    HP = NH // 2
    assert D == 64 and S % 512 == 0 and E % P == 0
    S_t = S // P          # seq tiles (16)
    E_t = E // P          # embed tiles (8)
    EH = NH * D           # own hidden dims (512)
    CH = 512              # q chunk
    spc = CH // P         # subtiles per chunk (4)
    n_ch = S // CH        # chunks (4)
    Lq = S // 2           # output rows per core
    f32 = mybir.dt.float32
    bf16 = mybir.dt.bfloat16
    Exp = mybir.ActivationFunctionType.Exp
    Ident = mybir.ActivationFunctionType.Identity
    scale = 1.0 / float(np.sqrt(E))
    groups = [[2 * i, 2 * i + 1] for i in range(n_cores // 2)]

    nc = bacc.Bacc(
        "TRN2", target_bir_lowering=False, debug=False, num_devices=n_cores
    )

    xT_d = nc.dram_tensor("xT", [E, S], bf16, kind="ExternalInput").ap()
    wqT_d = nc.dram_tensor("wqT", [E, EH], bf16, kind="ExternalInput").ap()
    wkT_d = nc.dram_tensor("wkT", [E, EH], bf16, kind="ExternalInput").ap()
    # full hidden rows x my 512 output columns (e-split output projection)
    woT_d = nc.dram_tensor("woT", [E, EH], bf16, kind="ExternalInput").ap()
    bo_d = nc.dram_tensor("bo", [P, EH // P], f32, kind="ExternalInput").ap()
    consts_d = nc.dram_tensor("consts", [P, 2 * P], bf16,
                              kind="ExternalInput").ap()
    # transposed output: my 512 e-columns for ALL queries
    out_d = nc.dram_tensor("out", [EH, S], bf16, kind="ExternalOutput").ap()

    with tile.TileContext(nc) as tc, ExitStack() as ctx:
        main = ctx.enter_context(tc.tile_pool(name="main", bufs=1))
        expp = ctx.enter_context(tc.tile_pool(name="expp", bufs=3))
        stgp = ctx.enter_context(tc.tile_pool(name="stgp", bufs=2))
        ostp = ctx.enter_context(tc.tile_pool(name="ostp", bufs=3))
        dram = ctx.enter_context(tc.tile_pool(name="dram", bufs=1,
                                              space="DRAM"))

        xTs = [main.tile([P, S], bf16, tag=f"xT{e}", name=f"xT{e}")
               for e in range(E_t)]
        wqf = main.tile([P, E_t, EH], bf16)
        wkf = main.tile([P, E_t, EH], bf16)
        wo = main.tile([P, E_t, EH], bf16)
        qn = main.tile([P, S_t, NH * (D + 1)], bf16)
        kT = main.tile([P, HP, S], bf16)
        qT = main.tile([P, HP, S], bf16)
        attnT = main.tile([P, HP, S], bf16)
        # all 16 heads' attn (own + peer, head order 0..15), via AllGather
        attnF = main.tile([P, 2 * HP, S], bf16)
        consts = main.tile([P, 2 * P], bf16)
        bo_sb = main.tile([P, EH // P], f32)
        ones128 = main.tile([1, P], bf16)
        wsrc = main.tile([P, 256], bf16)

        bxi = [dram.tile([HP * P, CH], bf16, tag=f"bxi{qc}", name=f"bxi{qc}")
               for qc in range(n_ch)]
        bxo = [dram.tile([2 * HP * P, CH], bf16, tag=f"bxo{qc}",
                         name=f"bxo{qc}") for qc in range(n_ch)]

        nc.vector.memset(ones128, 1.0)
        nc.vector.memset(wsrc, 0.0)

        ident = consts[:, 0:P]
        tri = consts[:, P:2 * P]
        qn4 = qn.rearrange("p t (h c) -> p t h c", c=D + 1)
        xT_r = xT_d.rearrange("(t p) s -> p t s", p=P)
        wq_r = wqT_d.rearrange("(t p) d -> p t d", p=P)
        wk_r = wkT_d.rearrange("(t p) d -> p t d", p=P)
        wo_r = woT_d.rearrange("(t p) e -> p t e", p=P)

        # ---- DMA issue order: first-needed first ----
        for e in range(E_t):
            nc.sync.dma_start(out=wqf[:, e, :], in_=wq_r[:, e, :])
        for e in range(E_t):
            nc.sync.dma_start(out=xTs[e][:, 0:CH], in_=xT_r[:, e, 0:CH])
        for e in range(E_t):
            nc.sync.dma_start(out=wkf[:, e, :], in_=wk_r[:, e, :])
        nc.sync.dma_start(out=consts, in_=consts_d)
        for sc in range(1, n_ch):
            for e in range(E_t):
                nc.sync.dma_start(
                    out=xTs[e][:, sc * CH:(sc + 1) * CH],
                    in_=xT_r[:, e, sc * CH:(sc + 1) * CH],
                )
        nc.sync.dma_start(out=bo_sb, in_=bo_d)
        for cp in range(E_t):
            nc.sync.dma_start(out=wo[:, cp, :], in_=wo_r[:, cp, :])

        # ---- PE warmup: keep HAM at 8/8 while the initial DMAs stream ----
        with tc.tile_pool(name="wps", bufs=1, space="PSUM") as wpsp:
            wdst = wpsp.tile([P, 256], f32)
            for _ in range(80):
                nc.tensor.matmul(wdst, wsrc[:, 0:P], wsrc, start=True,
                                 stop=True)

        pproj = ctx.enter_context(
            tc.tile_pool(name="pproj", bufs=2, space="PSUM"))
        psc = ctx.enter_context(
            tc.tile_pool(name="psc", bufs=2, space="PSUM"))
        pav = ctx.enter_context(
            tc.tile_pool(name="pav", bufs=1, space="PSUM"))

        # ---- task groups ----
        def qn_group(st):
            ps = pproj.tile([P, CH], f32, tag="ps", name="ps")
            for e in range(E_t):
                nc.tensor.matmul(
                    ps,
                    xTs[e][:, st * P:(st + 1) * P],
                    wqf[:, e, :],
                    start=(e == 0),
                    stop=(e == E_t - 1),
                )
            nc.vector.tensor_copy(
                out=qn4[:, st, :, 0:D],
                in_=ps.rearrange("p (h c) -> p h c", c=D),
            )
            nc.gpsimd.memset(qn4[:, st, :, D:D + 1], 1.0)

        def k_group(hp, sc):
            ps = pproj.tile([P, CH], f32, tag="ps", name="ps")
            for e in range(E_t):
                nc.tensor.matmul(
                    ps,
                    wkf[:, e, hp * P:(hp + 1) * P],
                    xTs[e][:, sc * CH:(sc + 1) * CH],
                    start=(e == 0),
                    stop=(e == E_t - 1),
                )
            nc.vector.tensor_copy(out=kT[:, hp, sc * CH:(sc + 1) * CH],
                                  in_=ps)

        def t_group(hp, qc):
            # transpose qn d-blocks of 4 seq tiles into qT for one head pair
            for st in range(4 * qc, 4 * qc + 4):
                ps = pproj.tile([P, CH], f32, tag="ps", name="ps")
                pt = ps.bitcast(bf16)
                nc.tensor.transpose(
                    pt[0:D, 0:P], qn4[:, st, 2 * hp, 0:D], ident)
                nc.tensor.transpose(
                    pt[D:P, 0:P], qn4[:, st, 2 * hp + 1, 0:D], ident)
                nc.vector.tensor_copy(
                    out=qT[:, hp, st * P:(st + 1) * P], in_=pt[:, 0:P])

        def x_group(qc):
            # exchange q-chunk qc of attnT with the pair peer via AllGather;
            # both contributions land in attnF in fixed head order 0..15, so
            # the program stays uniform. Post-CC DMAs ride the gpsimd queue.
            nc.sync.dma_start(
                out=bxi[qc].rearrange("(hp p) q -> p hp q", p=P),
                in_=attnT[:, :, qc * CH:(qc + 1) * CH])
            nc.gpsimd.collective_compute(
                "AllGather", mybir.AluOpType.bypass,
                replica_groups=groups,
                ins=[bxi[qc].opt()], outs=[bxo[qc].opt()],
            )
            nc.gpsimd.dma_start(
                out=attnF[:, :, qc * CH:(qc + 1) * CH],
                in_=bxo[qc].rearrange("(s p) q -> p s q", p=P))

        def o_group(qc, et):
            # output projection for q-chunk qc, e-tile et of my 512 columns,
            # over all 16 heads (attnF), bias added on the ACT-engine evict
            ps = pproj.tile([P, CH], f32, tag="ps", name="ps")
            for cp in range(2 * HP):
                nc.tensor.matmul(
                    ps,
                    wo[:, cp, et * P:(et + 1) * P],
                    attnF[:, cp, qc * CH:(qc + 1) * CH],
                    start=(cp == 0),
                    stop=(cp == 2 * HP - 1),
                )
            ot = ostp.tile([P, CH], bf16, tag="ot")
            nc.scalar.activation(out=ot, in_=ps, func=Ident,
                                 bias=bo_sb[:, et:et + 1])
            nc.sync.dma_start(
                out=out_d[et * P:(et + 1) * P, qc * CH:(qc + 1) * CH],
                in_=ot)

        # ---- prefix: projections needed by the qc0 attention units ----
        for st in range(4):
            qn_group(st)
        for hp in range(HP):
            k_group(hp, 0)
        for hp in range(HP):
            t_group(hp, 0)

        # ---- attention, qc-outer ----
        pending = []

        def flush_pending(use_pav=False):
            for (php, pqc, stgs) in pending:
                for half in range(2):
                    if use_pav:
                        rb = pav.tile([P, CH], f32,
                                      tag=("pvA" if half == 0 else "pvB"),
                                      name="rb")
                    else:
                        rb = pproj.tile([P, CH], f32, tag="ps", name="rb")
                    nc.tensor.matmul(rb, ones128[0:1, :], stgs[half],
                                     start=True, stop=True)
                    rcp = stgp.tile([P, CH], f32, tag="rbs", bufs=1)
                    nc.vector.reciprocal_approx_fast(out=rcp, in_=rb)
                    dst = attnT[half * D:(half + 1) * D, php,
                                pqc * CH:(pqc + 1) * CH]
                    nc.vector.tensor_tensor(out=dst, in0=dst,
                                            in1=rcp[half * D:(half + 1) * D,
                                                    :],
                                            op=mybir.AluOpType.mult)
            pending.clear()

        # attention chunk order 0,2,1,3: each chunk's AllGather exchange is
        # emitted one stream after its normalization and consumed (by the
        # output projection) one stream later still, hiding the ~16us
        # collective latency; only qc3's exchange is tail-serial.
        qorder = [0, 2, 1, 3]
        NE = EH // P  # my output e-tiles (4)
        stream_tasks = {
            0: ([lambda st=st: qn_group(st) for st in range(4, 16)]
                + [lambda hp=hp, sc=sc: k_group(hp, sc)
                   for sc in (1, 2) for hp in range(HP)]
                + [lambda hp=hp: t_group(hp, 2) for hp in range(HP)]),
            2: ([lambda: x_group(0)]
                + [lambda hp=hp: t_group(hp, 1) for hp in range(HP)]),
            1: ([lambda: x_group(2)]
                + [lambda hp=hp: t_group(hp, 3) for hp in range(HP)]
                + [lambda hp=hp: k_group(hp, 3) for hp in range(HP)]
                + [lambda et=et: o_group(0, et) for et in range(NE)]),
            3: ([lambda: x_group(1)]
                + [lambda et=et: o_group(2, et) for et in range(NE)]),
        }

        for qc in qorder:
            tasks = stream_tasks[qc]
            ti = 0
            t_max = 4 * qc + 3
            tstart = 5 if qc == 3 else 3
            slots = HP * max(1, t_max + 1 - tstart)
            tspace = max(1, slots // max(1, len(tasks)))
            for hp in range(HP):
                hA, hB = 2 * hp, 2 * hp + 1
                pvA = pav.tile([P, CH], f32, tag="pvA")
                pvB = pav.tile([P, CH], f32, tag="pvB")
                # software pipeline: attnV for unit t is emitted during unit
                # t+1, after the next scores+exp have been issued, so the PE
                # never sits in the scores->exp->attnV dependency chain.
                pipe = None

                def attn_v(ex, t, qoff):
                    nc.tensor.matmul(
                        pvA[0:D + 1, qoff:CH],
                        qn[:, t, hA * (D + 1):(hA + 1) * (D + 1)],
                        ex[:, 0, qoff:CH],
                        start=(t == 0),
                        stop=(t == t_max),
                    )
                    nc.tensor.matmul(
                        pvB[0:D + 1, qoff:CH],
                        qn[:, t, hB * (D + 1):(hB + 1) * (D + 1)],
                        ex[:, 1, qoff:CH],
                        start=(t == 0),
                        stop=(t == t_max),
                    )

                for t in range(t_max + 1):
                    if t == 2:
                        flush_pending()
                    jloc = max(0, t - 4 * qc)
                    qoff = jloc * P
                    sc_t = psc.tile([P, 2, CH], f32, tag="sc_t")
                    nc.tensor.matmul(
                        sc_t[:, 0, qoff:CH],
                        kT[0:D, hp, t * P:(t + 1) * P],
                        qT[0:D, hp, qc * CH + qoff:(qc + 1) * CH],
                        start=True, stop=True,
                    )
                    nc.tensor.matmul(
                        sc_t[:, 1, qoff:CH],
                        kT[D:P, hp, t * P:(t + 1) * P],
                        qT[D:P, hp, qc * CH + qoff:(qc + 1) * CH],
                        start=True, stop=True,
                    )
                    ex = expp.tile([P, 2, CH], bf16)
                    nc.scalar.activation(
                        out=ex[:, :, qoff:CH],
                        in_=sc_t[:, :, qoff:CH],
                        func=Exp,
                        scale=scale,
                    )
                    if t >= 4 * qc:
                        # frontier 128-block: triu multiply (vector, NOT
                        # gpsimd: the collectives block the gpsimd queue)
                        for h2 in range(2):
                            nc.vector.tensor_mul(
                                out=ex[:, h2, qoff:qoff + P],
                                in0=ex[:, h2, qoff:qoff + P],
                                in1=tri,
                            )
                    if pipe is not None:
                        attn_v(*pipe)
                    pipe = (ex, t, qoff)
                    if (t >= tstart and (t - tstart) % tspace == 0
                            and ti < len(tasks)):
                        tasks[ti]()
                        ti += 1
                attn_v(*pipe)
                # evict unnormalized attn + rowsum row; queue normalization
                stgs = []
                for pv, half in ((pvA, 0), (pvB, 1)):
                    stg = stgp.tile([1, CH], bf16, tag="stg", bufs=2)
                    nc.vector.tensor_copy(out=stg, in_=pv[D:D + 1, :])
                    nc.vector.tensor_copy(
                        out=attnT[half * D:(half + 1) * D, hp,
                                  qc * CH:(qc + 1) * CH],
                        in_=pv[0:D, :],
                    )
                    stgs.append(stg)
                pending.append((hp, qc, stgs))
            while ti < len(tasks):
                tasks[ti]()
                ti += 1

        # flush the last pending normalization into spare pav banks, then
        # tail: qc3 exchange; qc1's output projection and warm matmuls fill
        # the PE while the last AllGather is in flight
        wdst2 = psc.tile([P, 2, CH], f32, tag="sc_t")
        for _ in range(4):
            nc.tensor.matmul(wdst2[:, 0, 0:256], wsrc[:, 0:P], wsrc,
                             start=True, stop=True)
        flush_pending(use_pav=True)
        x_group(n_ch - 1)
        for et in range(EH // P):
            o_group(1, et)
        wdst3 = psc.tile([P, 2, CH], f32, tag="sc_t", name="wdst3")
        for _ in range(60):
            nc.tensor.matmul(wdst3[:, 0, 0:256], wsrc[:, 0:P], wsrc,
                             start=True, stop=True)
        for et in range(EH // P):
            o_group(n_ch - 1, et)

    nc.finalize()
    return nc


def _prep_inputs(x, Wk, Wq, Wo, bo, n_cores=NCORES):
    """Per-core input maps: batch = c//2, head half = c%2 (all bf16).

    wq/wk columns select the core's 8 heads; wo columns select the core's
    512 OUTPUT dims (e-split outproj over all 16 heads via the exchange).
    """
    b, s, e = x.shape
    P = 128
    EH = e // 2
    wqT = np.ascontiguousarray(Wq.T).astype(BF16)
    wkT = np.ascontiguousarray(Wk.T).astype(BF16)
    woT = np.ascontiguousarray(Wo.T).astype(BF16)
    consts = np.concatenate(
        [np.eye(P, dtype=np.float32),
         np.triu(np.ones((P, P), dtype=np.float32))], axis=1).astype(BF16)
    in_maps = []
    for c in range(n_cores):
        bi, hh = c // 2, c % 2
        xT = np.ascontiguousarray(x[bi].T).astype(BF16)
        bo_col = np.ascontiguousarray(
            bo[hh * EH:(hh + 1) * EH].reshape(EH // P, P).T
        ).astype(np.float32)
        in_maps.append({
            "xT": xT,
            "wqT": np.ascontiguousarray(wqT[:, hh * EH:(hh + 1) * EH]),
            "wkT": np.ascontiguousarray(wkT[:, hh * EH:(hh + 1) * EH]),
            "woT": np.ascontiguousarray(woT[:, hh * EH:(hh + 1) * EH]),
            "bo": bo_col,
            "consts": consts,
        })
    return in_maps


def kernel(x, Wk, Wq, Wv, Wo, bo):
    from concourse import bass_utils

    x = np.asarray(x, dtype=np.float32)
    Wk = np.asarray(Wk, dtype=np.float32)
    Wq = np.asarray(Wq, dtype=np.float32)
    Wo = np.asarray(Wo, dtype=np.float32)
    bo = np.asarray(bo, dtype=np.float32)
    b, s, e = x.shape
    key = (s, e, H)
    if key not in _CACHE:
        _CACHE[key] = _build_program(s, e, H)
    nc = _CACHE[key]
    in_maps = _prep_inputs(x, Wk, Wq, Wo, bo)
    res = bass_utils.run_bass_kernel_spmd(nc, in_maps, list(range(NCORES)))
    out = np.empty((b, s, e), dtype=np.float32)
    EH = e // 2
    for c in range(NCORES):
        bi, hh = c // 2, c % 2
        oc = np.asarray(res.results[c]["out"], dtype=np.float32)  # [EH, S]
        out[bi, :, hh * EH:(hh + 1) * EH] = oc.T
    return out


if __name__ == "__main__":
    nc = _build_program(S, E, H)
    print("built ok")
